# revision 18
# baseline (speedup 1.0000x reference)
"""Multi-head self-attention kernel for Trainium2, load-balanced over 8 NeuronCores.

Problem: B=8, S=1024, IN_DIM=D_MODEL=768, H=12, DK=64.
  q/k/v = Q @ W{q,k,v}.T + b   -> [b, H, s, dk]
  scores = exp(q k^T / 8) * key_mask ; attn = scores / (sum + 1e-8)
  out = attn @ v -> [b, s, 768]

Load balancing: the key mask means batch b only needs n_sk(b) = ceil(length[b]/128)
key tiles.  Work unit = (batch, head-pair) "job" costing n_sk(b) sk-tiles.  The 48
jobs are assigned to 8 cores x 6 group slots with minimized slot capacities (Hall
feasibility), so all cores run an identical program (SPMD) while per-group data
(which batch's Q^T, which pair's weight slices, masks) is staged host-side.  Raw
softmax numerators + denominators are emitted ([1024, 2x65] per group); the host
does the final divide + scatter (reference softmax is raw exp, so this is exact).

Per group g (capacity L): qT = Wq_pair^T-proj of its batch (full 1024 q cols),
kT only L*128 key cols held as two zero-padded per-head tiles (so every matmul is
K=128 -- no PE tile-mode switches), v [sk, 2x65] with ones column (rowsum trick),
exp fused with mask bias on ACT, ctx psum [128, 65] per (head, q-tile) accumulated
over sk, DVE-evacuated into one [128, 16*65] staging tile, one DMA per group.
Software pipeline: scores(g) interleaves ctx(g-1), vproj(g), and qkproj(g+1).
"""

import functools
import sys
import types

import numpy as np

B, S, IN_DIM, D_MODEL, H = 8, 1024, 768, 768, 12
DK = D_MODEL // H
NCORES = 8
NKT = IN_DIM // 128   # 6 contraction tiles
NPAIR = D_MODEL // 128  # 6 head pairs
NST = S // 128        # 8 s-tiles
NG = 6                # groups per core
MASK_BIAS = -60.0


def _install_shims():
    """antenv.axon_hooks shim (for NTFF tracing) + Tile drain-wait splitting
    (this walrus build accepts only one sync-wait command per Drain/CTRL)."""
    if 'antenv.axon_hooks' not in sys.modules:
        mod = types.ModuleType('antenv.axon_hooks')
        mod._hook = None
        mod.set_axon_ntff_profile_hook = lambda h: setattr(mod, '_hook', h)
        mod.get_axon_ntff_profile_hook = lambda: mod._hook
        sys.modules['antenv.axon_hooks'] = mod
        try:
            import antenv
            antenv.axon_hooks = mod
            from trn_agent_boot.trn_boot import _ntff_profile_via_ctypes
            mod.set_axon_ntff_profile_hook(
                _ntff_profile_via_ctypes('/opt/axon/libaxon_pjrt.so'))
        except Exception:
            pass

    import concourse.tile as tile
    if getattr(tile.TileContext, '_drain_patched', False):
        return
    from concourse.vector_clock import ScopedClock, VectorClock

    def _patched_drain_and_barrier(self, tick_clock, wait_clock):
        nc = self.nc
        gvec = tick_clock.global_clock
        n = len(gvec)
        for i in range(n):
            t = gvec[i]
            if t <= 0:
                continue
            v = [0] * n
            v[i] = t
            nop = nc.sync.nop(nofuse=True, hint="drain_wait_split")
            wait_clock.add_sem_waits(nop.ins, ScopedClock({None: VectorClock(v)}))
        # The per-proc NOPs above carry every wait (SP queue is in-order),
        # so the drain itself needs none.
        nc.sync.drain()
        nc.all_engine_barrier()
        assert self.sems is not None
        popped = nc._tile_sem_poison_stack.pop()
        assert popped is self._sem_poison
        nc.clear_and_free_semaphores(list(self.sems.allocated().values()))
        nc.all_engine_barrier()

    tile.TileContext._drain_and_barrier = _patched_drain_and_barrier

    # This walrus build accepts at most ONE sync-wait command per engine
    # instruction: split extra waits onto non-fusable NOPs emitted just
    # before the instruction on the same engine queue.
    import bass_rust
    import concourse.mybir as mybir
    _orig_lower = tile.TileContext._lower_ordered_insts

    def _split_waits_then_lower(self, ordered):
        nc = self.nc
        for bbname, insts in ordered.items():
            need = any(
                i.sync_info is not None and i.sync_info.on_wait
                and len(i.sync_info.on_wait) > 1
                for i in insts)
            if not need:
                continue
            out = []
            for inst in insts:
                si = inst.sync_info
                if si is not None and si.on_wait and len(si.on_wait) > 1:
                    waits = list(si.on_wait)
                    for w in waits[:-1]:
                        nop = mybir.InstNoOp(
                            name=nc.get_next_instruction_name(), ins=[], outs=[])
                        nop.engine = inst.engine
                        nop.bass_nofuse = True
                        nop.sync_info = bass_rust.SyncInfo(
                            on_wait=[w], on_update=[])
                        out.append(nop)
                    inst.sync_info = bass_rust.SyncInfo(
                        on_wait=[waits[-1]],
                        on_update=list(si.on_update or []))
                out.append(inst)
            insts[:] = out
        return _orig_lower(self, ordered)

    tile.TileContext._lower_ordered_insts = _split_waits_then_lower
    tile.TileContext._drain_patched = True


def _best_caps(sizes):
    """Minimal per-core slot capacities (NG slots, same on all 8 cores) such
    that the 48 jobs (sizes desc) fit slots with cap >= size (Hall)."""
    sizes = sorted(sizes, reverse=True)

    def feasible(caps):
        slots = sorted((c for c in caps for _ in range(NCORES)), reverse=True)
        return all(s >= j for j, s in zip(sizes, slots))

    caps = [sizes[m * NCORES] for m in range(NG)]
    improved = True
    while improved:
        improved = False
        for m in range(NG):
            if caps[m] > 1:
                trial = list(caps)
                trial[m] -= 1
                if feasible(trial):
                    caps = trial
                    improved = True
    return tuple(sorted(caps, reverse=True))


@functools.lru_cache(maxsize=None)
def _build_program(caps: tuple, use_bias: bool):
    import concourse.bass as bass
    import concourse.tile as tile
    import concourse.mybir as mybir
    from contextlib import ExitStack

    f32 = mybir.dt.float32
    bf16 = mybir.dt.bfloat16
    EXP = mybir.ActivationFunctionType.Exp
    LMAX = max(caps)

    nc = bass.Bass("TRN2", enable_partition_id=False)
    qt_d = nc.dram_tensor("qt", (NG, IN_DIM, S), bf16, kind="ExternalInput")
    wqk_d = nc.dram_tensor("wqk", (NG, IN_DIM, 256), bf16, kind="ExternalInput")
    wv_d = nc.dram_tensor("wv", (NG, IN_DIM, 128), bf16, kind="ExternalInput")
    mb_d = nc.dram_tensor("mb", (128, NG, NST), f32, kind="ExternalInput")
    if use_bias:
        bqk_d = nc.dram_tensor("bqk", (1, NG, 256), bf16, kind="ExternalInput")
        bv_d = nc.dram_tensor("bv", (1, NG, 128), bf16, kind="ExternalInput")
    # per group: 16 quarters of [128, 65] = (hl, sq) numerator(64)+denom(1)
    out_d = nc.dram_tensor("out", (NG, 128, 16 * 65), f32, kind="ExternalOutput")

    with tile.TileContext(nc) as tc, ExitStack() as ctx:
        const = ctx.enter_context(tc.tile_pool(name="const", bufs=1))
        qtpool = ctx.enter_context(tc.tile_pool(name="qt", bufs=2))
        wpool = ctx.enter_context(tc.tile_pool(name="w", bufs=2))
        qkpool = ctx.enter_context(tc.tile_pool(name="qk", bufs=2))
        vpool = ctx.enter_context(tc.tile_pool(name="v", bufs=2))
        prpool = ctx.enter_context(tc.tile_pool(name="pr", bufs=1))
        stpool = ctx.enter_context(tc.tile_pool(name="st", bufs=2))
        pj = ctx.enter_context(tc.tile_pool(name="pj", bufs=2, space="PSUM"))
        sc = ctx.enter_context(tc.tile_pool(name="sc", bufs=3, space="PSUM"))

        mb_sb = const.tile([128, NG, NST], f32)
        # dummy activation up front: pulls ACT_TABLE_LOAD into the DMA-wait
        # window instead of delaying the first real exp
        warm = const.tile([1, 8], f32)
        warm2 = const.tile([1, 8], f32)
        nc.vector.memset(warm, 0.0)
        nc.scalar.activation(out=warm2, in_=warm, func=EXP, scale=1.0)
        if use_bias:
            ones_sb = const.tile([1, 512], bf16)
            nc.vector.memset(ones_sb, 1.0)
            bqk_sb = const.tile([1, NG, 256], bf16)
            nc.sync.dma_start(out=bqk_sb, in_=bqk_d[:, :, :])
            bv_sb = const.tile([1, NG, 128], bf16)
            nc.sync.dma_start(out=bv_sb, in_=bv_d[:, :, :])

        pending = {}

        def prefetch(g, split_first=False):
            # weights first on the sync ring: the first q-proj matmul needs
            # wqk + qt chunk 0, so wqk must not queue behind bulk qt traffic
            wqk = wpool.tile([128, NKT, 256], bf16, tag="wqk", name=f"wqk{g}")
            src = wqk_d[g].rearrange("(k p) m -> p k m", p=128)
            qts = [qtpool.tile([128, S], bf16, tag=f"qt{k}", name=f"qt{g}_{k}")
                   for k in range(NKT)]
            wv = wpool.tile([128, NKT, 128], bf16, tag="wv", name=f"wv{g}")
            if split_first:
                # spread the critical first-group loads across both rings
                nc.sync.dma_start(out=wqk[:, 0:2, :], in_=src[:, 0:2, :])
                nc.gpsimd.dma_start(
                    out=qts[0], in_=qt_d[g, 0:128, :])
                nc.gpsimd.dma_start(out=wqk[:, 2:NKT, :], in_=src[:, 2:NKT, :])
            else:
                nc.sync.dma_start(out=wqk, in_=src)
            nc.sync.dma_start(
                out=wv, in_=wv_d[g].rearrange("(k p) m -> p k m", p=128))
            for k in range(NKT):
                if split_first and k == 0:
                    continue
                eng = nc.gpsimd if k % 2 == 0 else nc.sync
                eng.dma_start(out=qts[k], in_=qt_d[g, k * 128:(k + 1) * 128, :])
            pending[g] = (qts, wqk, wv)

        vs = {}
        kts = {}
        qTs = {}

        def qkproj_chunks(g):
            """Return list of emit-closures for group g's q/k projections."""
            L = caps[g]
            chunks = []

            def alloc():
                qTs[g] = qkpool.tile([128, S], bf16, tag="qT", name=f"qT{g}")
                kTA = qkpool.tile([128, LMAX * 128], bf16, tag="kTA",
                                  name=f"kTA{g}")
                kTB = qkpool.tile([128, LMAX * 128], bf16, tag="kTB",
                                  name=f"kTB{g}")
                if g < 2:
                    nc.vector.memset(kTA[64:128, :], 0.0)
                    nc.vector.memset(kTB[0:64, :], 0.0)
                kts[g] = (kTA, kTB)
                v_sb = vpool.tile([128, LMAX, 130], bf16, tag="v", name=f"v{g}")
                ones_dst = v_sb[:, 0:L, :].rearrange(
                    "p s (h x) -> p s h x", x=DK + 1)[:, :, :, DK:DK + 1]
                nc.vector.memset(ones_dst, 1.0)
                vs[g] = v_sb

            def qchunk(nch):
                qts, wqk, wv = pending[g]
                ps = pj.tile([128, 512], f32, tag="px", name=f"psq{g}_{nch}")
                for k in range(NKT):
                    nc.tensor.matmul(
                        ps,
                        lhsT=wqk[:, k, 0:128],
                        rhs=qts[k][:, nch * 512:(nch + 1) * 512],
                        start=(k == 0), stop=(k == NKT - 1 and not use_bias))
                if use_bias:
                    nc.tensor.matmul(
                        ps, lhsT=bqk_sb[0:1, g, 0:128],
                        rhs=ones_sb[0:1, 0:512], start=False, stop=True)
                nc.vector.tensor_copy(
                    out=qTs[g][:, nch * 512:(nch + 1) * 512], in_=ps)

            def kchunk(off, w):
                qts, wqk, wv = pending[g]
                ps = pj.tile([128, w], f32, tag="px", name=f"psk{g}_{off}")
                for k in range(NKT):
                    nc.tensor.matmul(
                        ps,
                        lhsT=wqk[:, k, 128:256],
                        rhs=qts[k][:, off:off + w],
                        start=(k == 0), stop=(k == NKT - 1 and not use_bias))
                if use_bias:
                    nc.tensor.matmul(
                        ps, lhsT=bqk_sb[0:1, g, 128:256],
                        rhs=ones_sb[0:1, 0:w], start=False, stop=True)
                kTA, kTB = kts[g]
                nc.vector.tensor_copy(
                    out=kTA[0:64, off:off + w], in_=ps[0:64, :])
                nc.vector.tensor_copy(
                    out=kTB[64:128, off:off + w], in_=ps[64:128, :])

            chunks.append(alloc)
            chunks.append(lambda: qchunk(0))
            chunks.append(lambda: qchunk(1))
            off = 0
            while off < L * 128:
                w = min(512, L * 128 - off)
                chunks.append(lambda off=off, w=w: kchunk(off, w))
                off += w
            return chunks

        def emit_vproj_sk(g, sk):
            qts, wqk, wv = pending[g]
            ps = pj.tile([128, 128], f32, tag="px", name=f"psv{g}_{sk}")
            for k in range(NKT):
                nc.tensor.matmul(
                    ps,
                    lhsT=qts[k][:, sk * 128:(sk + 1) * 128],
                    rhs=wv[:, k, :],
                    start=(k == 0), stop=(k == NKT - 1 and not use_bias))
            if use_bias:
                nc.tensor.matmul(
                    ps, lhsT=ones_sb[0:1, 0:128],
                    rhs=bv_sb[0:1, g, :], start=False, stop=True)
            dst = vs[g][:, sk, :].rearrange(
                "p (h x) -> p h x", x=DK + 1)[:, :, 0:DK]
            nc.vector.tensor_copy(
                out=dst, in_=ps.rearrange("p (h x) -> p h x", x=DK))

        probs = {}

        def emit_scores(g, sk):
            qT = qTs[g]
            kTA, kTB = kts[g]
            pss = []
            for hl in range(2):
                pss.append(sc.tile([128, S], f32, tag="sc",
                                   name=f"sc{g}_{sk}_{hl}"))
            for hl in range(2):
                kT = kTA if hl == 0 else kTB
                for nch in range(2):
                    nc.tensor.matmul(
                        pss[hl][:, nch * 512:(nch + 1) * 512],
                        lhsT=kT[:, sk * 128:(sk + 1) * 128],
                        rhs=qT[:, nch * 512:(nch + 1) * 512],
                        start=True, stop=True)
                pb = prpool.tile([128, S], bf16, tag=f"pb{g % 2}_{hl}_{sk}",
                                 name=f"pb{g}_{hl}_{sk}")
                nc.scalar.activation(
                    out=pb, in_=pss[hl], func=EXP,
                    bias=mb_sb[:, g, sk:sk + 1], scale=1.0 / np.sqrt(DK))
                probs[(g % 2, hl, sk)] = pb

        stages = {}

        def emit_ctx_piece(g, u):
            """u in 0..15: hl = u // 8, sq = u % 8."""
            L = caps[g]
            hl, sq = u // 8, u % 8
            if u == 0:
                stages[g] = stpool.tile([128, 16 * 65], f32, tag="st",
                                        name=f"st{g}")
            pc = pj.tile([128, 65], f32, tag="px", name=f"cx{g}_{u}")
            for sk in range(L):
                nc.tensor.matmul(
                    pc,
                    lhsT=probs[(g % 2, hl, sk)][:, sq * 128:(sq + 1) * 128],
                    rhs=vs[g][:, sk, hl * 65:(hl + 1) * 65],
                    start=(sk == 0), stop=(sk == L - 1))
            dst = stages[g][:, u * 65:(u + 1) * 65]
            if g == NG - 1 and u % 2 == 1:
                nc.scalar.copy(out=dst, in_=pc)
            else:
                nc.vector.tensor_copy(out=dst, in_=pc)
            if u == 7:
                nc.scalar.dma_start(
                    out=out_d[g][:, 0:8 * 65], in_=stages[g][:, 0:8 * 65])
            elif u == 15:
                nc.scalar.dma_start(
                    out=out_d[g][:, 8 * 65:], in_=stages[g][:, 8 * 65:])

        # ---- main pipeline (fill paced by estimated PE cost)
        # PE warmup during the initial DMA wait: ~4.5us of dummy matmuls so
        # the HAM clock-gate releases (1.2 -> 2.4 GHz) before real work
        wtile = const.tile([128, 512], bf16)
        nc.vector.memset(wtile, 0.0)
        for wi in range(10):
            # warm psums come from the sc pool: first real scores allocation
            # happens ~20us in, so no interference with early projections
            wp = sc.tile([128, 512], f32, tag="sc", name=f"warmmm{wi}")
            nc.tensor.matmul(wp, lhsT=wtile[:, 0:128], rhs=wtile,
                             start=True, stop=True)
        prefetch(0, split_first=True)
        nc.gpsimd.dma_start(out=mb_sb, in_=mb_d[:, :, :])
        chunks0 = qkproj_chunks(0)
        for ch in chunks0[:4]:   # alloc, q0, q1, k0 -> scores can start
            ch()
        prefetch(1)
        for g in range(NG):
            L = caps[g]
            fill = []  # (cost_ns, closure)
            CK = 6 * 260  # qk chunk cost
            if g == 0:
                for i, ch in enumerate(chunks0[4:]):
                    fill.append((CK, ch))
            nxt = qkproj_chunks(g + 1) if g + 1 < NG else []
            ctx_pieces = []
            if g > 0:
                Lp = caps[g - 1]
                ctx_pieces = [(Lp * 35 + 250,
                               (lambda u=u: emit_ctx_piece(g - 1, u)))
                              for u in range(16)]
            vp = [(6 * 60, (lambda sk=sk: emit_vproj_sk(g, sk)))
                  for sk in range(L)]
            nxtc = [(CK if i > 0 else 150, ch) for i, ch in enumerate(nxt)]
            a, b, c = len(nxtc), len(ctx_pieces), len(vp)
            n = max(a, b, c)
            for i in range(n):
                if i < a:
                    fill.append(nxtc[i])
                if i < b:
                    fill.append(ctx_pieces[i])
                if i < c:
                    fill.append(vp[i])
            if g + 2 < NG:
                fill.append((150, lambda g=g: prefetch(g + 2)))
            total = sum(f[0] for f in fill)
            fi = 0
            done = 0
            for sk in range(L):
                emit_scores(g, sk)
                tgt = total * (sk + 1) // L
                while fi < len(fill) and done < tgt:
                    done += fill[fi][0]
                    fill[fi][1]()
                    fi += 1
            while fi < len(fill):
                fill[fi][1]()
                fi += 1
        for u in range(16):
            emit_ctx_piece(NG - 1, u)

    return nc


TRACE = False
LAST_EXEC_NS = None
LAST_RES = None


def kernel(Q, length, Wq, bq, Wk, bk, Wv, bv):
    global LAST_EXEC_NS, LAST_RES
    _install_shims()
    from concourse.bass_utils import run_bass_kernel_spmd

    Q = np.asarray(Q, np.float32)
    length = np.asarray(length, np.int32)
    Wq, Wk, Wv = (np.asarray(w, np.float32) for w in (Wq, Wk, Wv))
    bq, bk, bv = (np.asarray(b, np.float32) for b in (bq, bk, bv))

    use_bias = bool(np.any(bq) or np.any(bk) or np.any(bv))

    # job list: (batch, pair) costing n_sk(batch)
    n_sk_b = [max(1, min(NST, -(-int(min(max(l, 0), S)) // 128)))
              for l in length]
    jobs = sorted(
        ((b, t) for b in range(B) for t in range(NPAIR)),
        key=lambda j: (-n_sk_b[j[0]], j[0], j[1]))
    assert len(jobs) == NCORES * NG
    caps_desc = _best_caps([n_sk_b[b] for (b, t) in jobs])
    # order: smallest first (fast pipeline ramp), big groups mid, small
    # near the end (tiny tail): desc [c0..c5] -> [c5, c0, c1, c2, c3, c4]
    order = (5, 0, 1, 2, 3, 4)
    caps = tuple(caps_desc[i] for i in order)
    # assign jobs (desc) to slots (desc cap, core-major) elementwise
    slot_order = sorted(
        ((m, c) for m in range(NG) for c in range(NCORES)),
        key=lambda s: (-caps[s[0]], s[1], s[0]))
    core_jobs = [[None] * NG for _ in range(NCORES)]
    for job, (m, c) in zip(jobs, slot_order):
        assert caps[m] >= n_sk_b[job[0]]
        core_jobs[c][m] = job

    import ml_dtypes
    bfl = ml_dtypes.bfloat16
    qt_all = np.ascontiguousarray(Q.transpose(0, 2, 1)).astype(bfl)  # [B,768,1024]
    wqT = np.ascontiguousarray(Wq.T).astype(bfl)   # [768, 768]
    wkT = np.ascontiguousarray(Wk.T).astype(bfl)
    wvT = np.ascontiguousarray(Wv.T).astype(bfl)
    j = np.arange(S)
    mb_all = np.where(j[None, :] < length[:, None], 0.0, MASK_BIAS).astype(np.float32)
    mb_all = np.ascontiguousarray(mb_all.reshape(B, NST, 128).transpose(0, 2, 1))

    nc = _build_program(caps, use_bias)
    in_maps = []
    for c in range(NCORES):
        cj = core_jobs[c]
        qt = np.stack([qt_all[b] for (b, t) in cj])                    # [NG,768,1024]
        wqk = np.stack([
            np.concatenate([wqT[:, t * 128:(t + 1) * 128],
                            wkT[:, t * 128:(t + 1) * 128]], axis=1)
            for (b, t) in cj])                                         # [NG,768,256]
        wv = np.stack([wvT[:, t * 128:(t + 1) * 128] for (b, t) in cj])
        mb = np.stack([mb_all[b] for (b, t) in cj], axis=1)            # [128,NG,NST]
        m = {"qt": qt, "wqk": wqk, "wv": wv, "mb": np.ascontiguousarray(mb)}
        if use_bias:
            bqf, bkf, bvf = (x.astype(np.float32) for x in (bq, bk, bv))
            m["bqk"] = np.stack([
                np.concatenate([bqf[t * 128:(t + 1) * 128],
                                bkf[t * 128:(t + 1) * 128]])
                for (b, t) in cj])[None].astype(bfl)
            m["bv"] = np.stack([bvf[t * 128:(t + 1) * 128]
                                for (b, t) in cj])[None].astype(bfl)
        in_maps.append(m)

    res = run_bass_kernel_spmd(
        nc, in_maps, core_ids=list(range(NCORES)), trace=TRACE)
    LAST_EXEC_NS = res.exec_time_ns
    LAST_RES = res

    out = np.zeros((B, S, D_MODEL), np.float32)
    for c in range(NCORES):
        raw = res.results[c]["out"]  # [NG, 128, 16*65]
        for m, (b, t) in enumerate(core_jobs[c]):
            quads = raw[m].reshape(128, 2, 8, 65)  # [p, hl, sq, x]
            for hl in range(2):
                arr = quads[:, hl].transpose(1, 0, 2).reshape(S, 65)
                num = arr[:, :DK]
                den = arr[:, DK:DK + 1]
                head = 2 * t + hl
                out[b, :, head * DK:(head + 1) * DK] = num / (den + 1e-8)
    return np.ascontiguousarray(out)
